# revision 61
# baseline (speedup 1.0000x reference)
"""GCN layer kernel for Trainium2 (8 NeuronCores, SPMD).

Computes: support = x @ W.T + b; agg[d] = sum_{e: dst[e]=d} val[e] * support[src[e]];
out = leaky_relu(agg, 0.2).

Strategy (dst-sharded, 6250 dst nodes / ~100K edges per core), single phase:
aggregate-then-transform. Since (A @ (x W.T)) == ((A @ x) W.T), each core
gathers full bf16 x rows (512B descriptors -- same DMA cost per descriptor
as a 256B support row would be, due to the sub-512B half-rate penalty) for
its edges and aggregates them per 128-dst block with one-hot S matmuls,
applying the tiny W transform once per block:

  For each dst block of 128 nodes (49 slots per core) and each <=128-edge
  chunk:
    S[slot, d] = val[slot] if dcol[slot] == d else 0   (DVE tensor_scalar,
                 bf16 in/out for the 2x DVE mode)
    ZT_h[f, d] += Xg[:, 128h:128h+128].T @ S           (PE, accumulated in
                 PSUM transposed so no PE-transpose is ever needed)
  Flush: ZT -> SBUF bf16 (Act + DVE, one half each), out.T[fo, d] =
  sum_h wT_h.T @ ZT_h (PE), leaky relu (Act scale-copy + DVE max), staged
  bf16 and written to HBM in three overlapping pieces.

Sharding: the 391 global 128-dst blocks are dealt to (core, slot) by sorted
edge count so each SPMD slot's eight blocks are similarly sized -- the
uniform per-slot capacities (cross-core maxima; one shared program runs on
all 8 cores) then hug the per-core need. Edges go to two gather streams by
int16 index range ("A" reads x rows [0, 32768), "B" reads [17408, 50176);
overlap rows are assigned to fill A's capacity). Each (slot, stream) is one
dma_gather call whose num_idxs is the exact 16-aligned slot capacity; the
last chunk may be partially filled and is K-sliced out of the matmuls.
Duplicate (src, block) edge pairs (~2% of edges) share one gather slot:
the second edge rides as (dcol2, val2) metadata applied by an extra S-build
on the block's first chunk. Metadata travels compressed (int8 dcol, bf16
val) and is widened once on DVE. The bias b is folded in via a per-block
rank-1 update (sum-of-val row times b) only when b is nonzero; the common
b == 0 case skips it entirely.
"""

import numpy as np
import ml_dtypes

N_NODES = 50000
N_EDGES = 800000
D_IN = 256
D_OUT = 64
NEG_SLOPE = 0.2
N_CORES = 8
NPC = N_NODES // N_CORES  # dst nodes per core
PAIR = 128  # dst block width (S matrix columns / psum partitions)
NPAIRS = -(-NPC // PAIR)  # dst blocks per core
XROWS = 50176  # x table rows (padded to multiple of 128)
SPLIT = 32768  # int16 index limit: A stream rows < SPLIT
BASE1 = XROWS - SPLIT  # B stream rows >= BASE1
GBUFS = 5  # gather tile buffer depth (per stream)
SBUFS = 8  # S-matrix buffer depth
ZTBUFS = 2  # psum ZT buffer depth (per half; psum tiles are bank-granular)
ZSBUFS = 3  # ZT sbuf staging buffer depth


IDXALIGN = 8  # per-call index-region alignment, in idx columns (x2B = 16B)


def _idx_layout(cap):
    """Aligned per-slot index-column layout for one stream.

    Each (slot, stream) dma_gather call reads its own wrapped index region;
    the gather ucode needs the region base aligned (arbitrary 16B offsets
    corrupt it), so each region starts at a multiple of IDXALIGN columns.
    Returns (ibase list, icols list, total columns).
    """
    ibase, icols = [], []
    pos = 0
    for c in cap:
        assert c % 16 == 0
        ibase.append(pos)
        icols.append(c // 16)
        pos += -(-(c // 16) // IDXALIGN) * IDXALIGN
    return ibase, icols, pos


def _build_schedule(edge_src, edge_dst, edge_val):
    """Pack edges into per-core per-stream slot arrays.

    Returns (streams, capA, capB, blocks) where streams[s] = (idx, dcol, val)
    arrays of shape [N_CORES, 128, nch_s], capA/capB are per-slot exact
    capacities (shared across cores; the compiled program is SPMD), and
    blocks[k][j] is the global 128-dst block processed by core k in slot j
    (used to reassemble the output on the host).

    Blocks are dealt to (core, slot) by sorted edge count so each slot's
    eight blocks are similarly sized: the uniform capacity (a cross-core
    max) then hugs the per-core need.
    """
    nblocks = -(-N_NODES // PAIR)  # 391 global dst blocks
    blk_bounds = np.searchsorted(
        edge_dst, np.arange(0, (NPAIRS * N_CORES + 1) * PAIR, PAIR)
    )
    blk_cnt = np.diff(blk_bounds)  # edges per global block (0 for dummies)
    # deal sorted blocks round-robin: octet j gets ranks [8j, 8j+8), biggest
    # octets first so the final slot processed holds the smallest blocks
    order = np.argsort(-blk_cnt, kind="stable")
    blocks = np.empty((N_CORES, NPAIRS), np.int64)
    for j in range(NPAIRS):
        blocks[:, j] = np.sort(order[j * N_CORES : (j + 1) * N_CORES])

    per_core = []
    cnt_a = np.zeros((N_CORES, NPAIRS), np.int64)
    cnt_f = np.zeros((N_CORES, NPAIRS), np.int64)
    cnt = np.zeros((N_CORES, NPAIRS), np.int64)
    for k in range(N_CORES):
        sel = [
            np.arange(blk_bounds[b], blk_bounds[b + 1]) for b in blocks[k]
        ]
        slot_of = np.concatenate(
            [np.full(len(ix), j) for j, ix in enumerate(sel)]
        )
        eidx = np.concatenate(sel)
        esrc = edge_src[eidx].astype(np.int64)
        edloc = edge_dst[eidx].astype(np.int64) - blocks[k][slot_of] * PAIR
        eval_ = edge_val[eidx]
        ep = slot_of
        # merge duplicate (src, block) edge pairs into one gather slot: the
        # second edge rides as (dcol2, val2) applied by an extra S-build on
        # the block's first chunk
        o = np.lexsort((esrc, ep))
        esrc, edloc, eval_, ep = esrc[o], edloc[o], eval_[o], ep[o]
        key = ep * (XROWS + 1) + esrc
        newgrp = np.empty(len(key), bool)
        newgrp[0] = True
        newgrp[1:] = key[1:] != key[:-1]
        gstart = np.zeros(len(key), np.int64)
        gid = np.cumsum(newgrp) - 1
        first_of_grp = np.where(newgrp)[0]
        within_g = np.arange(len(key)) - first_of_grp[gid]
        is_first = within_g % 2 == 0  # slot-defining edge
        # second edge of each pair attaches to the preceding slot
        src = esrc[is_first]
        dloc = edloc[is_first]
        val = eval_[is_first]
        p = ep[is_first]
        isdup = np.zeros(len(src), bool)
        att = np.where(~is_first)[0]  # attached edges (immediately follow)
        slot_of_edge = np.cumsum(is_first) - 1
        dcol2 = np.full(len(src), -1.0, np.float64)
        val2 = np.zeros(len(src), np.float64)
        dcol2[slot_of_edge[att]] = edloc[att]
        val2[slot_of_edge[att]] = eval_[att]
        isdup[slot_of_edge[att]] = True
        aonly = src < BASE1
        bonly = src >= SPLIT
        flex = ~aonly & ~bonly
        cnt_a[k] = np.bincount(p[aonly], minlength=NPAIRS)
        cnt_f[k] = np.bincount(p[flex], minlength=NPAIRS)
        cnt[k] = np.bincount(p, minlength=NPAIRS)
        per_core.append((src, dloc, val, p, aonly, flex, dcol2, val2, isdup))

    # near-exact uniform capacities per slot: A must fit every core's A-only
    # count; B then holds the rest (flex rows go to A first). Rounded up to
    # multiples of 16 -- the dma_gather index list is consumed in 16-wraps
    # and non-multiple-of-16 num_idxs corrupts/crashes the gather ucode.
    capA = -(-np.maximum(cnt_a.max(axis=0), 1) // 16) * 16
    a_k = np.minimum(cnt_a + cnt_f, capA[None, :])
    capB = -(-np.maximum((cnt - a_k).max(axis=0), 1) // 16) * 16
    nchA = -(-capA // 128)
    nchB = -(-capB // 128)

    baseA = np.zeros(NPAIRS + 1, np.int64)
    baseA[1:] = np.cumsum(nchA)
    baseB = np.zeros(NPAIRS + 1, np.int64)
    baseB[1:] = np.cumsum(nchB)
    nch = [int(baseA[-1]), int(baseB[-1])]
    base = [baseA, baseB]

    out = {
        s: (
            np.zeros((N_CORES, 128, nch[s]), np.int16),
            np.zeros((N_CORES, 128, nch[s]), np.float32),
            np.zeros((N_CORES, 128, nch[s]), np.float32),
        )
        for s in range(2)
    }
    d2 = {
        s: (
            np.full((N_CORES, 128, NPAIRS), -1.0, np.float32),
            np.zeros((N_CORES, 128, NPAIRS), np.float32),
        )
        for s in range(2)
    }
    for k in range(N_CORES):
        src, dloc, val, p, aonly, flex, dcol2, val2, isdup = per_core[k]
        # flexible slots to A, as many as fit A's capacity (B then fits too,
        # by construction of the capacities)
        take = np.minimum(cnt_f[k], capA - cnt_a[k])
        start_f = np.zeros(NPAIRS, np.int64)
        start_f[1:] = np.cumsum(cnt_f[k])[:-1]
        fw = p[flex]
        ordinal = np.arange(flex.sum(), dtype=np.int64) - start_f[fw]
        flex_to_a = np.zeros(len(src), bool)
        flex_to_a[np.where(flex)[0]] = ordinal < take[fw]
        in_a = aonly | flex_to_a
        for s, mask in enumerate([in_a, ~in_a]):
            sel = np.where(mask)[0]
            # dup slots first within each block so they land in chunk 0
            sel = sel[np.lexsort((~isdup[sel], p[sel]))]
            ps = p[sel]
            counts = np.bincount(ps, minlength=NPAIRS)
            assert np.all(counts <= (capA if s == 0 else capB)), (
                k, s, counts.max())
            start_p = np.zeros(NPAIRS, np.int64)
            start_p[1:] = np.cumsum(counts)[:-1]
            within = np.arange(len(sel), dtype=np.int64) - start_p[ps]
            assert np.all(within[isdup[sel]] < 128)
            slot_c = base[s][ps] + within // 128
            slot_p = within % 128
            idx, dcol, valx = out[s]
            assert dloc.min() >= 0 and dloc.max() < PAIR
            idx[k, slot_p, slot_c] = (src[sel] - s * BASE1).astype(np.int16)
            dcol[k, slot_p, slot_c] = dloc[sel].astype(np.float32)
            valx[k, slot_p, slot_c] = val[sel]
            dm = isdup[sel]
            d2[s][0][k, slot_p[dm], ps[dm]] = dcol2[sel[dm]].astype(np.float32)
            d2[s][1][k, slot_p[dm], ps[dm]] = val2[sel[dm]].astype(np.float32)

    # wrap indices per (slot, stream) call region in the aligned layout
    caps = [capA, capB]
    streams = {}
    for s in range(2):
        idx, dcol, valx = out[s]
        ibase, icols, ncols = _idx_layout(list(caps[s]))
        idxw = np.zeros((N_CORES, 16, ncols), np.int16)
        for p in range(NPAIRS):
            lo = base[s][p]
            cap16 = int(caps[s][p])
            seg = idx[:, :, lo : lo + (cap16 + 127) // 128]
            flat = seg.transpose(0, 2, 1).reshape(N_CORES, -1)[:, :cap16]
            idxw[:, :, ibase[p] : ibase[p] + icols[p]] = (
                flat.reshape(N_CORES, icols[p], 16).swapaxes(1, 2)
            )
        streams[s] = (idxw, dcol, valx, d2[s][0], d2[s][1])
    return streams, [int(v) for v in capA], [int(v) for v in capB], blocks


def _build_program(capA, capB, has_bias):
    import concourse.bacc as bacc
    import concourse.mybir as mybir
    from concourse import tile

    F32 = mybir.dt.float32
    BF16 = mybir.dt.bfloat16
    I16 = mybir.dt.int16
    caps = [capA, capB]
    nchs = [[-(-c // 128) for c in cap] for cap in caps]
    base = []
    for s in range(2):
        b = [0]
        for v in nchs[s]:
            b.append(b[-1] + v)
        base.append(b)
    nch = [base[0][-1], base[1][-1]]
    ilay = [_idx_layout(capA), _idx_layout(capB)]  # (ibase, icols, ncols)

    nc = bacc.Bacc(None, target_bir_lowering=False, debug=False)
    x_d = nc.dram_tensor("xr", [XROWS, D_IN], BF16, kind="ExternalInput")
    wT_d = nc.dram_tensor("wT", [D_IN, D_OUT], BF16, kind="ExternalInput")
    iota_d = nc.dram_tensor("iota", [128, PAIR], BF16, kind="ExternalInput")
    idx_d = [
        nc.dram_tensor(f"idx{s}", [128, ilay[s][2]], I16, kind="ExternalInput")
        for s in range(2)
    ]
    dcol_d = [
        nc.dram_tensor(f"dcol{s}", [128, nch[s]], mybir.dt.int8, kind="ExternalInput")
        for s in range(2)
    ]
    val_d = [
        nc.dram_tensor(f"val{s}", [128, nch[s]], BF16, kind="ExternalInput")
        for s in range(2)
    ]
    dcol2_d = [
        nc.dram_tensor(f"dcol2{s}", [128, NPAIRS], mybir.dt.int8, kind="ExternalInput")
        for s in range(2)
    ]
    val2_d = [
        nc.dram_tensor(f"val2{s}", [128, NPAIRS], BF16, kind="ExternalInput")
        for s in range(2)
    ]
    if has_bias:
        bb_d = nc.dram_tensor("bb", [1, D_OUT], BF16, kind="ExternalInput")
        ident_d = nc.dram_tensor("ident", [128, 128], BF16, kind="ExternalInput")
    y_d = nc.dram_tensor("y", [D_OUT, NPAIRS * PAIR], BF16, kind="ExternalOutput")

    with tile.TileContext(nc) as tc:
        with (
            tc.tile_pool(name="const", bufs=1) as cpool,
            tc.tile_pool(name="stage", bufs=1) as stpool,
            tc.tile_pool(name="gath", bufs=GBUFS) as gpool,
            tc.tile_pool(name="smat", bufs=SBUFS) as spool,
            tc.tile_pool(name="ztsb", bufs=ZSBUFS) as zspool,
            tc.tile_pool(name="psum", bufs=ZTBUFS, space="PSUM") as pspool,
        ):
            wt_t = cpool.tile([128, 2, D_OUT], BF16)
            iota_t = cpool.tile([128, PAIR], BF16)
            idx_t = [
                cpool.tile([128, ilay[s][2]], I16, name=f"idx{s}t", tag=f"idx{s}")
                for s in range(2)
            ]
            dcol8_t = [
                cpool.tile([128, nch[s]], mybir.dt.int8, name=f"dcol8{s}t",
                           tag=f"dcol8{s}")
                for s in range(2)
            ]
            valh_t = [
                cpool.tile([128, nch[s]], BF16, name=f"valh{s}t", tag=f"valh{s}")
                for s in range(2)
            ]
            # the is_equal tensor_scalar needs f32 scalars: metadata travels
            # over HBM compressed (int8 dcol / bf16 val) and is widened once
            dcol_t = [
                cpool.tile([128, nch[s]], F32, name=f"dcol{s}t", tag=f"dcol{s}")
                for s in range(2)
            ]
            val_t = [
                cpool.tile([128, nch[s]], F32, name=f"val{s}t", tag=f"val{s}")
                for s in range(2)
            ]
            dcol28_t = [
                cpool.tile([128, NPAIRS], mybir.dt.int8, name=f"dcol28{s}t",
                           tag=f"dcol28{s}")
                for s in range(2)
            ]
            val2h_t = [
                cpool.tile([128, NPAIRS], BF16, name=f"val2h{s}t",
                           tag=f"val2h{s}")
                for s in range(2)
            ]
            dcol2_t = [
                cpool.tile([128, NPAIRS], F32, name=f"dcol2{s}t", tag=f"dcol2{s}")
                for s in range(2)
            ]
            val2_t = [
                cpool.tile([128, NPAIRS], F32, name=f"val2{s}t", tag=f"val2{s}")
                for s in range(2)
            ]
            for s in range(2):
                nc.sync.dma_start(out=idx_t[s][:], in_=idx_d[s][:])
            for s in range(2):
                nc.sync.dma_start(out=dcol8_t[s][:], in_=dcol_d[s][:])
                nc.sync.dma_start(out=valh_t[s][:], in_=val_d[s][:])
                nc.sync.dma_start(out=dcol28_t[s][:], in_=dcol2_d[s][:])
                nc.sync.dma_start(out=val2h_t[s][:], in_=val2_d[s][:])
            for s in range(2):
                nc.vector.tensor_copy(dcol_t[s][:], dcol8_t[s][:])
                nc.vector.tensor_copy(val_t[s][:], valh_t[s][:])
                nc.vector.tensor_copy(dcol2_t[s][:], dcol28_t[s][:])
                nc.vector.tensor_copy(val2_t[s][:], val2h_t[s][:])
            wT_v = wT_d.rearrange("(kk p) f -> p kk f", p=128)
            nc.sync.dma_start(out=wt_t[:], in_=wT_v)
            nc.sync.dma_start(out=iota_t[:], in_=iota_d[:])
            if has_bias:
                bb_t = cpool.tile([1, D_OUT], BF16)
                ident_t = cpool.tile([128, 128], BF16)
                ones_t = cpool.tile([128, 1], BF16)
                nc.sync.dma_start(out=bb_t[:], in_=bb_d[:])
                nc.sync.dma_start(out=ident_t[:], in_=ident_d[:])
                nc.vector.memset(ones_t[:], 1.0)

            x_base = [x_d[0:SPLIT, :], x_d[BASE1:XROWS, :]]

            def gather_pair(s, p, seg):
                # gather calls per (block, stream): num_idxs is the exact
                # slot capacity, so the last chunk may be partially filled
                # (its unwritten tail is excluded from matmuls via K-slicing).
                # `seg` chunks per call: one call per block normally; the last
                # block is split into small segments to shorten the drain tail.
                lo = base[s][p]
                nch_p = nchs[s][p]
                segs = []
                for c0 in range(0, nch_p, seg):
                    c1 = min(c0 + seg, nch_p)
                    n_idx = min(caps[s][p] - c0 * 128, (c1 - c0) * 128)
                    gt = gpool.tile(
                        [128, c1 - c0, D_IN], BF16,
                        name=f"g{s}_{p}_{c0}", tag=f"G{s}",
                    )
                    # the gather ucode reads exactly cdiv(n_idx, 16) index
                    # columns from this call's aligned region
                    i0 = ilay[s][0][p] + c0 * 8
                    nc.gpsimd.dma_gather(
                        gt[:, :, :],
                        x_base[s],
                        idx_t[s][:, i0 : i0 + (n_idx + 15) // 16],
                        n_idx,
                        n_idx,
                        D_IN,
                        single_packet=False,
                    )
                    segs.append((c0, gt))
                return segs

            def seg_lookup(segs, i):
                for c0, gt in reversed(segs):
                    if i >= c0:
                        return gt, i - c0
                raise AssertionError

            out_stage = stpool.tile([D_OUT, NPAIRS, PAIR], BF16)
            for p in range(NPAIRS):
                zt = [
                    pspool.tile([128, PAIR], F32, name=f"zt{h}", tag=f"zt{h}")
                    for h in range(2)
                ]
                if has_bias:
                    rs = pspool.tile([128, 1], F32, name="rs", tag="rs", bufs=1)
                seq = []
                for s in range(2):
                    for i in range(nchs[s][p]):
                        kk = min(caps[s][p] - i * 128, 128)
                        seq.append((s, i, kk))
                seg = 10**9
                gts = [gather_pair(0, p, seg), gather_pair(1, p, seg)]
                for j, (s, i, kk) in enumerate(seq):
                    c = base[s][p] + i
                    gt, off = seg_lookup(gts[s], i)
                    s_t = spool.tile([128, PAIR], BF16, tag="S")
                    nc.vector.tensor_scalar(
                        s_t[:], iota_t[:],
                        dcol_t[s][:, c : c + 1], val_t[s][:, c : c + 1],
                        op0=mybir.AluOpType.is_equal,
                        op1=mybir.AluOpType.mult,
                    )
                    if i == 0:
                        # chunk 0 holds the merged duplicate-source slots:
                        # apply the second edges' one-hot on top
                        s2_t = spool.tile([128, PAIR], BF16, tag="S2", bufs=2)
                        nc.vector.tensor_scalar(
                            s2_t[:], iota_t[:],
                            dcol2_t[s][:, p : p + 1], val2_t[s][:, p : p + 1],
                            op0=mybir.AluOpType.is_equal,
                            op1=mybir.AluOpType.mult,
                        )
                        sm_t = spool.tile([128, PAIR], BF16, tag="SM", bufs=2)
                        nc.vector.tensor_tensor(
                            sm_t[:], s_t[:], s2_t[:], op=mybir.AluOpType.add
                        )
                        s_t = sm_t
                    first = j == 0
                    last = j == len(seq) - 1
                    for h in range(2):
                        nc.tensor.matmul(
                            zt[h][:],
                            gt[:kk, off, 128 * h : 128 * (h + 1)],
                            s_t[:kk, :],
                            start=first,
                            stop=last,
                        )
                    if has_bias:
                        nc.tensor.matmul(
                            rs[:], s_t[:], ones_t[:], start=first, stop=last
                        )
                # flush block p (the two ZT copies go to different engines so
                # they run in parallel on the critical drain path)
                zts = zspool.tile([128, 2, PAIR], BF16, tag="zts")
                nc.scalar.activation(
                    zts[:, 0, :], zt[0][:], mybir.ActivationFunctionType.Copy
                )
                nc.vector.tensor_copy(zts[:, 1, :], zt[1][:])
                outp = pspool.tile([D_OUT, PAIR], F32, name="outp", tag="op")
                for h in range(2):
                    nc.tensor.matmul(
                        outp[:],
                        wt_t[:, h, :],
                        zts[:, h, :],
                        start=(h == 0),
                        stop=(h == 1 and not has_bias),
                    )
                if has_bias:
                    rs_sb = zspool.tile([128, 1], BF16, tag="rssb")
                    nc.scalar.activation(
                        rs_sb[:], rs[:], mybir.ActivationFunctionType.Copy
                    )
                    rsT = pspool.tile([1, 128], BF16, name="rsT", tag="rsT", bufs=1)
                    nc.tensor.transpose(rsT[:], rs_sb[:], ident_t[:])
                    rsT_sb = zspool.tile([1, 128], BF16, tag="rsTsb")
                    nc.scalar.activation(
                        rsT_sb[:], rsT[:], mybir.ActivationFunctionType.Copy
                    )
                    nc.tensor.matmul(
                        outp[:], bb_t[:], rsT_sb[:], start=False, stop=True
                    )
                tmp = zspool.tile([D_OUT, PAIR], mybir.dt.float32, tag="tmp")
                if p == NPAIRS - 1:
                    # final block: keep the leaky chain on one engine (DVE)
                    # to avoid a cross-engine semaphore hop on the critical
                    # program-drain path
                    nc.vector.tensor_copy(tmp[:], outp[:])
                    nc.vector.scalar_tensor_tensor(
                        out_stage[:, p, :], tmp[:], NEG_SLOPE, tmp[:],
                        op0=mybir.AluOpType.mult,
                        op1=mybir.AluOpType.max,
                    )
                else:
                    nc.scalar.activation(
                        tmp[:], outp[:], mybir.ActivationFunctionType.Copy,
                        scale=NEG_SLOPE,
                    )
                    nc.vector.tensor_tensor(
                        out_stage[:, p, :], tmp[:], outp[:],
                        op=mybir.AluOpType.max,
                    )
                if p == NPAIRS - 9:
                    # overlap most of the output write with the last blocks
                    nc.sync.dma_start(
                        out=y_d[:, : (p + 1) * PAIR],
                        in_=out_stage[:, : p + 1, :],
                    )
                elif p == NPAIRS - 2:
                    nc.sync.dma_start(
                        out=y_d[:, (NPAIRS - 8) * PAIR : (NPAIRS - 1) * PAIR],
                        in_=out_stage[:, NPAIRS - 8 : NPAIRS - 1, :],
                    )
            nc.sync.dma_start(
                out=y_d[:, (NPAIRS - 1) * PAIR :],
                in_=out_stage[:, NPAIRS - 1 :, :],
            )
    nc.compile()
    return nc


LAST_RESULTS = None  # BassKernelResults of the most recent run (for profiling)
LAST_NC = None  # compiled Bass module of the most recent run


def kernel(x, W, b, edge_src, edge_dst, edge_val):
    global LAST_RESULTS, LAST_NC
    from concourse import bass_utils

    x = np.asarray(x)
    W = np.asarray(W)
    b = np.asarray(b)
    edge_src = np.asarray(edge_src)
    edge_dst = np.asarray(edge_dst)
    edge_val = np.asarray(edge_val)

    streams, capA, capB, blocks = _build_schedule(edge_src, edge_dst, edge_val)
    has_bias = bool(np.any(b != 0))

    xr = np.zeros((XROWS, D_IN), ml_dtypes.bfloat16)
    xr[:N_NODES] = x.astype(ml_dtypes.bfloat16)
    wT = np.ascontiguousarray(W.T).astype(ml_dtypes.bfloat16)
    iota = np.tile(np.arange(PAIR, dtype=np.float32), (128, 1)).astype(
        ml_dtypes.bfloat16
    )

    nc = _build_program(capA, capB, has_bias)
    LAST_NC = nc

    shared = {"xr": xr, "wT": wT, "iota": iota}
    if has_bias:
        shared["bb"] = b.astype(ml_dtypes.bfloat16).reshape(1, D_OUT)
        shared["ident"] = np.eye(128, dtype=ml_dtypes.bfloat16)
    in_maps = []
    for k in range(N_CORES):
        m = dict(shared)
        for s in range(2):
            idxw, dcol, val, dcol2, val2 = streams[s]
            m[f"idx{s}"] = np.ascontiguousarray(np.tile(idxw[k], (8, 1)))
            m[f"dcol{s}"] = dcol[k].astype(np.int8)
            m[f"val{s}"] = val[k].astype(ml_dtypes.bfloat16)
            m[f"dcol2{s}"] = dcol2[k].astype(np.int8)
            m[f"val2{s}"] = val2[k].astype(ml_dtypes.bfloat16)
        in_maps.append(m)

    res = None
    for attempt in range(3):
        try:
            res = bass_utils.run_bass_kernel_spmd(
                nc, in_maps, core_ids=list(range(N_CORES))
            )
            break
        except Exception:
            # Transient NRT/axon execution failures have been observed; the
            # device recovers on a fresh dispatch. Re-raise on the last try.
            if attempt == 2:
                raise
    LAST_RESULTS = res
    out = np.zeros((NPAIRS * N_CORES * PAIR, D_OUT), np.float32)
    for k in range(N_CORES):
        yk = np.asarray(res.results[k]["y"]).astype(np.float32)  # [64, NPAIRS*128]
        for j in range(NPAIRS):
            blk = int(blocks[k][j])
            out[blk * PAIR : (blk + 1) * PAIR] = yk[:, j * PAIR : (j + 1) * PAIR].T
    return out[:N_NODES]


if __name__ == "__main__":
    pass
